# revision 5
# baseline (speedup 1.0000x reference)
"""GAT (2-layer, 4-head) Trainium2 Bass kernel, 8-core SPMD.

Strategy: partition dst nodes across 8 cores (6250 each). Per core, sort owned
dst by (lo-degree, hi-degree), group into 49 windows of 128 dst; dst = SBUF
partition lane, its incoming edges = slots along the free dim (window-uniform
slot counts, shared across cores so the single SPMD program fits all cores).
Per-edge source rows [h_fp16(256B) | a_src_f32(16B) | pad] are gathered from a
512B-stride HBM table via dma_gather (int16 idx => lo/hi table halves split at
32768). Softmax over incoming edges runs dst-major: logits/exp on DVE+ACT,
weighted message sum via broadcast-multiply + free-dim reduce on DVE.
Layer-1 table is built replicated (each core computes h1 = x@W1 for all
nodes); layer-2 table is an AllGather of per-shard h2 rows.
"""

import os
import numpy as np
from contextlib import ExitStack

import concourse.bass as bass
import concourse.tile as tile
from concourse import bacc, mybir
from concourse.bass_utils import run_bass_kernel_spmd

# problem constants (hardcoded per contest contract)
N = 50000
E = 1600000
HEADS = 4
HID = 32
INF = 128          # input feature dim == hidden dim HEADS*HID
OUTF = 8
NCORES = 8
NLOC = N // NCORES          # 6250 dst per core
WPC = (NLOC + 127) // 128   # 49 windows per core
NPAD = WPC * 128            # 6272
SPLIT = 32768               # int16 gather-index split point
ROWW = 256                  # fp16 words per table row (512 B)
NXP = ((N + 2047) // 2048) * 2048  # 50176: xT padded cols
TB2 = NCORES * NPAD         # 50176 rows in layer-2 table
DCAP = 24                   # slot-chunk for multiply/reduce working tile
NEG = -1e30

F32 = mybir.dt.float32
F16 = mybir.dt.float16
I16 = mybir.dt.int16

_CACHE = {}
LAST_RESULT = None
LAST_NC = None
LAST_IN_MAPS = None


# ----------------------------------------------------------------------------
# host-side graph preprocessing
# ----------------------------------------------------------------------------

def _host_prep(edge_index):
    srcs = np.concatenate([edge_index[0], np.arange(N)]).astype(np.int64)
    dsts = np.concatenate([edge_index[1], np.arange(N)]).astype(np.int64)
    ne = srcs.shape[0]

    core = dsts // NLOC
    dloc = dsts % NLOC

    lo1 = srcs < SPLIT
    deg_lo = np.bincount(dsts[lo1], minlength=N)
    deg_hi = np.bincount(dsts[~lo1], minlength=N)

    perms = []           # per core: sorted_pos -> orig local dst id
    pos = np.empty(N, np.int64)   # node -> position within its owner core
    for c in range(NCORES):
        lo_c = deg_lo[c * NLOC:(c + 1) * NLOC]
        hi_c = deg_hi[c * NLOC:(c + 1) * NLOC]
        p = np.lexsort((-hi_c, -(lo_c // 6)))  # coarse-lo buckets, hi sorted within
        perms.append(p)
        pos[c * NLOC + p] = np.arange(NLOC)

    t2row = (np.arange(N) // NLOC) * NPAD + pos  # node -> layer-2 table row

    wpos = pos[dsts]
    w_e = wpos // 128
    dpart_e = wpos % 128

    layers = {}
    for L, (sec, val) in enumerate([
        ((srcs >= SPLIT).astype(np.int64), np.where(srcs < SPLIT, srcs, srcs - SPLIT)),
        ((t2row[srcs] >= SPLIT).astype(np.int64),
         np.where(t2row[srcs] < SPLIT, t2row[srcs], t2row[srcs] - SPLIT)),
    ], start=1):
        key = ((core * WPC + w_e) * 2 + sec) * 128 + dpart_e
        order = np.argsort(key, kind="stable")
        ks = key[order]
        change = np.r_[True, ks[1:] != ks[:-1]]
        gid = np.cumsum(change) - 1
        startpos = np.flatnonzero(change)
        j_sorted = np.arange(ne) - startpos[gid]
        j = np.empty(ne, np.int64)
        j[order] = j_sorted

        cnt = np.bincount(key, minlength=NCORES * WPC * 2 * 128)
        cnt = cnt.reshape(NCORES, WPC, 2, 128)
        dsec = cnt.max(axis=(0, 3))          # [WPC, 2]
        dlo = dsec[:, 0].astype(np.int64)
        dhi = dsec[:, 1].astype(np.int64)
        cs_lo = np.r_[0, np.cumsum(dlo)]
        cs_hi = np.r_[0, np.cumsum(dhi)]
        cs_m = np.r_[0, np.cumsum(dlo + dhi)]
        tot_lo, tot_hi, tot_m = int(cs_lo[-1]), int(cs_hi[-1]), int(cs_m[-1])

        idx_lo = np.zeros((NCORES, max(tot_lo, 1) * 128), np.int16)
        idx_hi = np.zeros((NCORES, max(tot_hi, 1) * 128), np.int16)
        maskf = np.full((NCORES, 128, tot_m), NEG, np.float32)

        is_lo = sec == 0
        fpos_lo = (cs_lo[w_e[is_lo]] + j[is_lo]) * 128 + dpart_e[is_lo]
        idx_lo[core[is_lo], fpos_lo] = val[is_lo].astype(np.int16)
        fpos_hi = (cs_hi[w_e[~is_lo]] + j[~is_lo]) * 128 + dpart_e[~is_lo]
        idx_hi[core[~is_lo], fpos_hi] = val[~is_lo].astype(np.int16)

        mcol = cs_m[w_e] + j + np.where(is_lo, 0, dlo[w_e])
        maskf[core, dpart_e, mcol] = 0.0

        # keep pad dst rows (window WPC-1, pos>=NLOC) finite: open slot 0
        maskf[:, (NLOC % 128):, cs_m[WPC - 1]] = 0.0

        # rewrap idx arrays into dma_gather layout [128, 8*tot]
        def wrap(arr, dvec, csvec, tot):
            out = np.zeros((NCORES, 128, 8 * max(tot, 1)), np.int16)
            for w in range(WPC):
                d = int(dvec[w])
                if d == 0:
                    continue
                cs = int(csvec[w])
                blk = arr[:, cs * 128:(cs + d) * 128]          # [NC, d*128]
                blk = blk.reshape(NCORES, -1, 16).transpose(0, 2, 1)  # [NC,16,8d]
                out[:, :, 8 * cs: 8 * (cs + d)] = np.tile(blk, (1, 8, 1))
            return out

        layers[L] = dict(
            dlo=dlo, dhi=dhi, cs_lo=cs_lo, cs_hi=cs_hi, cs_m=cs_m,
            tot_lo=tot_lo, tot_hi=tot_hi, tot_m=tot_m,
            idx_lo=wrap(idx_lo, dlo, cs_lo, tot_lo),
            idx_hi=wrap(idx_hi, dhi, cs_hi, tot_hi),
            mask=maskf,
        )

    return layers, perms


def _fold_weights(W1, att_src1, att_dst1, b1, W2, att_src2, att_dst2, b2, Wout, bout):
    # device feature order is head-interleaved: dev k = c*4 + h <-> ref h*32 + c
    perm = np.array([h * 32 + c for c in range(HID) for h in range(HEADS)])

    def vec(att):  # [HEADS, HID] -> [128, 4] fold in dev space
        z = np.zeros((INF, HEADS), np.float32)
        k = np.arange(INF)
        z[k, k % HEADS] = att[k % HEADS, k // HEADS]
        return z

    W1d = W1[:, perm]
    rhs1 = np.concatenate([W1d, W1d @ vec(att_src1)], axis=1).astype(np.float32)  # [128,132]
    wdx1 = (W1d @ vec(att_dst1)).astype(np.float32)                               # [128,4]
    W2d = W2[perm][:, perm]
    rhs2 = np.concatenate([W2d, W2d @ vec(att_src2), W2d @ vec(att_dst2)], axis=1).astype(np.float32)
    woutd = Wout[perm].astype(np.float32)                                          # [128,8]
    b1t = np.tile(b1[perm].astype(np.float32), (128, 1))                           # [128,128]
    bf = (b2 @ Wout + bout).astype(np.float32)
    bft = np.tile(bf, (128, 1))                                                    # [128,8]
    return rhs1, wdx1, rhs2, woutd, b1t, bft


# ----------------------------------------------------------------------------
# device program
# ----------------------------------------------------------------------------

def _edge_phase(nc, tc, pools, meta, table, idx_in, mask_in, ad_tile, den_tile, aggn_cb):
    """Per-window gather + softmax + weighted aggregation. aggn_cb(w, aggn_ap)
    consumes the normalized [128,128] aggregate of window w."""
    dlo, dhi = meta["dlo"], meta["dhi"]
    cs_lo, cs_hi, cs_m = meta["cs_lo"], meta["cs_hi"], meta["cs_m"]
    gpool, wpool, spool = pools["g"], pools["w"], pools["s"]
    idx_lo_in, idx_hi_in = idx_in

    mpool = pools["m"]
    GRP = 8  # windows per metadata load group (few big DMAs instead of 6/window)
    grp_of = {}
    grp_tiles = {}
    for g0 in range(0, WPC, GRP):
        g1 = min(g0 + GRP, WPC)
        nlo = int(cs_lo[g1] - cs_lo[g0])
        nhi = int(cs_hi[g1] - cs_hi[g0])
        nm = int(cs_m[g1] - cs_m[g0])
        ilo_g = mpool.tile([128, 8 * max(nlo, 1)], I16, tag="ilog")
        if nlo:
            nc.sync.dma_start(ilo_g[:, 0:8 * nlo],
                              idx_lo_in[:, 8 * int(cs_lo[g0]): 8 * int(cs_lo[g1])])
        ihi_g = mpool.tile([128, 8 * max(nhi, 1)], I16, tag="ihig")
        if nhi:
            nc.sync.dma_start(ihi_g[:, 0:8 * nhi],
                              idx_hi_in[:, 8 * int(cs_hi[g0]): 8 * int(cs_hi[g1])])
        mk_g = mpool.tile([128, nm], F32, tag="mkg")
        nc.sync.dma_start(mk_g[:], mask_in[:, int(cs_m[g0]): int(cs_m[g1])])
        for w in range(g0, g1):
            grp_of[w] = g0
            grp_tiles[w] = (ilo_g, ihi_g, mk_g)

    for w in range(WPC):
        d_lo, d_hi = int(dlo[w]), int(dhi[w])
        d = d_lo + d_hi
        assert d >= 1
        g0 = grp_of[w]
        ilo_g, ihi_g, mk_g = grp_tiles[w]
        olo = 8 * int(cs_lo[w] - cs_lo[g0])
        ohi = 8 * int(cs_hi[w] - cs_hi[g0])
        om = int(cs_m[w] - cs_m[g0])

        abl = os.environ.get("GAT_ABL", "")
        xg = gpool.tile([128, d, ROWW], F16, tag="xg")
        if "nogather" in abl:
            nc.vector.memset(xg[:, :, 128:136].bitcast(F32), 0.5)
        elif "half" in abl:
            nc.vector.memset(xg[:, :, 128:136].bitcast(F32), 0.5)
            xh = gpool.tile([128, d, 128], F16, tag="xh")
            if d_lo:
                nc.gpsimd.dma_gather(xh[:, 0:d_lo, :], table[0:SPLIT, 0:128],
                                     ilo_g[:, olo:olo + 8 * d_lo],
                                     128 * d_lo, 128 * d_lo, 128, elem_step=ROWW,
                                     single_packet="sp" in abl)
            if d_hi:
                nc.gpsimd.dma_gather(xh[:, d_lo:d, :], table[SPLIT:, 0:128],
                                     ihi_g[:, ohi:ohi + 8 * d_hi],
                                     128 * d_hi, 128 * d_hi, 128, elem_step=ROWW,
                                     single_packet="sp" in abl)
        else:
            sp = "sp" in abl
            if d_lo:
                nc.gpsimd.dma_gather(xg[:, 0:d_lo, :], table[0:SPLIT, :],
                                     ilo_g[:, olo:olo + 8 * d_lo],
                                     128 * d_lo, 128 * d_lo, ROWW, single_packet=sp)
            if d_hi:
                nc.gpsimd.dma_gather(xg[:, d_lo:d, :], table[SPLIT:, :],
                                     ihi_g[:, ohi:ohi + 8 * d_hi],
                                     128 * d_hi, 128 * d_hi, ROWW, single_packet=sp)

        mk = mk_g[:, om:om + d]

        # logits: lrelu(a_s[src] + a_d[dst]) + mask  -> exp
        as_view = xg[:, :, 128:136].bitcast(F32)                      # [128,d,4]
        lp = spool.tile([128, d, 4], F32, tag="lp")
        nc.vector.tensor_add(lp[:], as_view,
                             ad_tile[:, 4 * w:4 * w + 4].unsqueeze(1).broadcast_to([128, d, 4]))
        nc.vector.tensor_add(lp[:], lp[:], mk.unsqueeze(2).broadcast_to([128, d, 4]))
        ll = spool.tile([128, d, 4], F32, tag="ll")
        nc.vector.scalar_tensor_tensor(ll[:], lp[:], 0.2, lp[:],
                                       mybir.AluOpType.mult, mybir.AluOpType.max)
        ew = spool.tile([128, d, 4], F32, tag="ew")
        nc.scalar.activation(ew[:], ll[:], mybir.ActivationFunctionType.Exp)

        # denominators for this window
        nc.vector.tensor_reduce(den_tile[:, 4 * w:4 * w + 4], ew[:].transpose([0, 2, 1]),
                                mybir.AxisListType.X, mybir.AluOpType.add)

        # weighted message aggregation, slot-chunked
        agg = spool.tile([128, 128], F32, tag="agg")
        if "nomul" in abl:
            nc.vector.memset(agg[:], 1.0)
        first = "nomul" in abl
        for j0 in ([] if "nomul" in abl else range(0, d, DCAP)):
            dc = min(DCAP, d - j0)
            wm = wpool.tile([128, dc, 128], F32, tag="wm")
            xv = xg[:, j0:j0 + dc, 0:128].rearrange("p j (c h) -> p j c h", h=HEADS)
            eb = ew[:, j0:j0 + dc, :].unsqueeze(2).broadcast_to([128, dc, HID, HEADS])
            nc.vector.tensor_mul(wm[:].rearrange("p j (c h) -> p j c h", h=HEADS), xv, eb)
            if first:
                nc.vector.tensor_reduce(agg[:], wm[:].transpose([0, 2, 1]),
                                        mybir.AxisListType.X, mybir.AluOpType.add)
                first = False
            else:
                ac = spool.tile([128, 128], F32, tag="aggc")
                nc.vector.tensor_reduce(ac[:], wm[:].transpose([0, 2, 1]),
                                        mybir.AxisListType.X, mybir.AluOpType.add)
                nc.vector.tensor_add(agg[:], agg[:], ac[:])

        rec = spool.tile([128, 4], F32, tag="rec")
        nc.vector.reciprocal(rec[:], den_tile[:, 4 * w:4 * w + 4])
        aggn = spool.tile([128, 128], F32, tag="aggn")
        nc.vector.tensor_mul(aggn[:].rearrange("p (c h) -> p c h", h=HEADS),
                             agg[:].rearrange("p (c h) -> p c h", h=HEADS),
                             rec[:].unsqueeze(1).broadcast_to([128, HID, HEADS]))
        aggn_cb(w, aggn)


def _build_program(meta1, meta2):
    nc = bacc.Bacc("TRN2", num_devices=NCORES)

    xT = nc.dram_tensor("xT", [128, NXP], F32, kind="ExternalInput")
    xs = nc.dram_tensor("xs", [128, NPAD], F32, kind="ExternalInput")
    rhs1_h = nc.dram_tensor("rhs1", [128, 132], F32, kind="ExternalInput")
    wdx1_h = nc.dram_tensor("wdx1", [128, 4], F32, kind="ExternalInput")
    rhs2_h = nc.dram_tensor("rhs2", [128, 136], F32, kind="ExternalInput")
    wout_h = nc.dram_tensor("woutd", [128, 8], F32, kind="ExternalInput")
    b1t_h = nc.dram_tensor("b1t", [128, 128], F32, kind="ExternalInput")
    bft_h = nc.dram_tensor("bft", [128, 8], F32, kind="ExternalInput")
    ident_h = nc.dram_tensor("ident", [128, 128], F32, kind="ExternalInput")

    i1lo = nc.dram_tensor("i1lo", [128, 8 * max(meta1["tot_lo"], 1)], I16, kind="ExternalInput")
    i1hi = nc.dram_tensor("i1hi", [128, 8 * max(meta1["tot_hi"], 1)], I16, kind="ExternalInput")
    i2lo = nc.dram_tensor("i2lo", [128, 8 * max(meta2["tot_lo"], 1)], I16, kind="ExternalInput")
    i2hi = nc.dram_tensor("i2hi", [128, 8 * max(meta2["tot_hi"], 1)], I16, kind="ExternalInput")
    m1 = nc.dram_tensor("m1", [128, meta1["tot_m"]], F32, kind="ExternalInput")
    m2 = nc.dram_tensor("m2", [128, meta2["tot_m"]], F32, kind="ExternalInput")

    outy = nc.dram_tensor("outy", [NPAD, OUTF], F32, kind="ExternalOutput")

    T1 = nc.dram_tensor("T1", [NXP, ROWW], F16, kind="Internal")
    AGIN = nc.dram_tensor("AGIN", [NPAD, ROWW], F16, kind="Internal")
    T2 = nc.dram_tensor("T2", [TB2, ROWW], F16, kind="Internal")

    with ExitStack() as ctx:
        tc = ctx.enter_context(tile.TileContext(nc))
        cpool = ctx.enter_context(tc.tile_pool(name="consts", bufs=1))
        pers = ctx.enter_context(tc.tile_pool(name="pers", bufs=1))
        gpool = ctx.enter_context(tc.tile_pool(name="gather", bufs=2))
        wpool = ctx.enter_context(tc.tile_pool(name="work", bufs=2))
        spool = ctx.enter_context(tc.tile_pool(name="small", bufs=3))
        mpool = ctx.enter_context(tc.tile_pool(name="meta", bufs=2))
        pspool = ctx.enter_context(tc.tile_pool(name="ps", bufs=3, space="PSUM"))
        ptpool = ctx.enter_context(tc.tile_pool(name="pt", bufs=2, space="PSUM"))
        pools = {"g": gpool, "w": wpool, "s": spool, "m": mpool}

        def const(h, shape, dtype=F32, tag=None):
            t = cpool.tile(shape, dtype, tag=tag)
            nc.sync.dma_start(t[:], h[:])
            return t

        rhs1_t = const(rhs1_h, [128, 132], tag="rhs1")
        wdx1_t = const(wdx1_h, [128, 4], tag="wdx1")
        rhs2_t = const(rhs2_h, [128, 136], tag="rhs2")
        wout_t = const(wout_h, [128, 8], tag="wout")
        b1t_t = const(b1t_h, [128, 128], tag="b1t")
        bft_t = const(bft_h, [128, 8], tag="bft")
        ident_t = const(ident_h, [128, 128], tag="identc")
        xs_t = const(xs, [128, NPAD], tag="xs")

        ad1 = pers.tile([128, 4 * WPC], F32)
        ad2 = pers.tile([128, 4 * WPC], F32)
        den1 = pers.tile([128, 4 * WPC], F32)
        den2 = pers.tile([128, 4 * WPC], F32)
        fin = pers.tile([128, OUTF * WPC], F32)

        # ---- phase A: build T1 = [h1_fp16 | a_s1_f32 | pad] for all nodes ----
        XBLK = 2048
        for b0 in range(0, NXP, XBLK):
            xt_t = gpool.tile([128, XBLK], F32, tag="xg")
            nc.sync.dma_start(xt_t[:], xT[:, b0:b0 + XBLK])
            rb = wpool.tile([128, (XBLK // 128) * 128], F32, tag="wm")
            rb16 = rb[:].bitcast(F16)
            for k in range(XBLK // 128):
                ps = pspool.tile([128, 132], F32)
                nc.tensor.matmul(ps[:], xt_t[:, k * 128:(k + 1) * 128], rhs1_t[:],
                                 start=True, stop=True)
                nc.scalar.copy(rb16[:, k * 256:k * 256 + 128], ps[:, 0:128])
                nc.vector.tensor_copy(rb[:, k * 128 + 64:k * 128 + 68], ps[:, 128:132])
                nc.vector.memset(rb[:, k * 128 + 68:k * 128 + 128], 0.0)
            nc.sync.dma_start(
                T1[b0:b0 + XBLK, :].rearrange("(a p) r -> p a r", p=128),
                rb16.rearrange("p (a r) -> p a r", a=XBLK // 128))

        # a_d1 for owned (sorted) nodes
        for w in range(WPC):
            ps4 = ptpool.tile([128, 4], F32, tag="pss")
            nc.tensor.matmul(ps4[:], xs_t[:, w * 128:(w + 1) * 128], wdx1_t[:],
                             start=True, stop=True)
            nc.vector.tensor_copy(ad1[:, 4 * w:4 * w + 4], ps4[:])

        stop = os.environ.get("GAT_STOP", "full")
        if stop == "a":
            nc.vector.memset(fin[:], 0.0)
        # ---- phase B: layer-1 edge phase + layer-2 row build ----
        def tail1(w, aggn):
            t = spool.tile([128, 128], F32, tag="t1t")
            nc.vector.tensor_add(t[:], aggn[:], b1t_t[:])
            mn = spool.tile([128, 128], F32, tag="t1m")
            nc.vector.tensor_scalar_min(mn[:], t[:], 0.0)
            ex = spool.tile([128, 128], F32, tag="t1e")
            nc.scalar.activation(ex[:], mn[:], mybir.ActivationFunctionType.Exp)
            x2 = spool.tile([128, 128], F32, tag="t1x")
            nc.vector.scalar_tensor_tensor(x2[:], t[:], 0.0, ex[:],
                                           mybir.AluOpType.max, mybir.AluOpType.add)
            nc.vector.tensor_scalar_sub(x2[:], x2[:], 1.0)
            x2t_ps = ptpool.tile([128, 128], F32, tag="tr")
            nc.tensor.transpose(x2t_ps[:], x2[:], ident_t[:])
            x2t = spool.tile([128, 128], F32, tag="t1xt")
            nc.vector.tensor_copy(x2t[:], x2t_ps[:])
            ps = pspool.tile([128, 136], F32)
            nc.tensor.matmul(ps[:], x2t[:], rhs2_t[:], start=True, stop=True)
            rowt = spool.tile([128, 128], F32, tag="rowt")
            nc.scalar.copy(rowt[:].bitcast(F16)[:, 0:128], ps[:, 0:128])
            nc.vector.tensor_copy(rowt[:, 64:68], ps[:, 128:132])
            nc.vector.memset(rowt[:, 68:128], 0.0)
            nc.vector.tensor_copy(ad2[:, 4 * w:4 * w + 4], ps[:, 132:136])
            nc.sync.dma_start(
                AGIN[w * 128:(w + 1) * 128, :].rearrange("(a p) r -> p a r", p=128),
                rowt[:].bitcast(F16).rearrange("p (a r) -> p a r", a=1))

        if stop != "a":
            _edge_phase(nc, tc, pools, meta1, T1, (i1lo, i1hi), m1, ad1, den1, tail1)
        if stop in ("l1", "a"):
            if stop == "l1":
                nc.vector.memset(fin[:], 0.0)
        else:
            # ---- all-gather layer-2 table ----
            nc.gpsimd.collective_compute(
                "AllGather", mybir.AluOpType.bypass,
                replica_groups=[list(range(NCORES))],
                ins=[AGIN[:].opt()], outs=[T2[:].opt()])

        # ---- phase C: layer-2 edge phase + final projection ----
        def tail2(w, aggn):
            at_ps = ptpool.tile([128, 128], F32, tag="tr")
            nc.tensor.transpose(at_ps[:], aggn[:], ident_t[:])
            at = spool.tile([128, 128], F32, tag="t2at")
            nc.vector.tensor_copy(at[:], at_ps[:])
            ps8 = ptpool.tile([128, 8], F32, tag="pss")
            nc.tensor.matmul(ps8[:], at[:], wout_t[:], start=True, stop=True)
            nc.vector.tensor_add(fin[:, OUTF * w:OUTF * (w + 1)], ps8[:], bft_t[:])

        if stop == "ag":
            nc.vector.memset(fin[:], 0.0)
        if stop == "full":
            _edge_phase(nc, tc, pools, meta2, T2, (i2lo, i2hi), m2, ad2, den2, tail2)

        nc.sync.dma_start(outy[:].rearrange("(a p) r -> p a r", p=128),
                          fin[:].rearrange("p (a r) -> p a r", a=WPC))

    nc.compile()
    return nc


# ----------------------------------------------------------------------------
# entry point
# ----------------------------------------------------------------------------

def kernel(x, edge_index, W1, att_src1, att_dst1, b1, W2, att_src2, att_dst2,
           b2, Wout, bout):
    global LAST_RESULT
    x = np.asarray(x, np.float32)
    edge_index = np.asarray(edge_index)

    ck = hash(edge_index.tobytes())
    if ck not in _CACHE:
        layers, perms = _host_prep(edge_index)
        nc = _build_program(layers[1], layers[2])
        _CACHE.clear()
        _CACHE[ck] = (layers, perms, nc)
    layers, perms, nc = _CACHE[ck]
    meta1, meta2 = layers[1], layers[2]

    rhs1, wdx1, rhs2, woutd, b1t, bft = _fold_weights(
        np.asarray(W1, np.float32), np.asarray(att_src1, np.float32),
        np.asarray(att_dst1, np.float32), np.asarray(b1, np.float32),
        np.asarray(W2, np.float32), np.asarray(att_src2, np.float32),
        np.asarray(att_dst2, np.float32), np.asarray(b2, np.float32),
        np.asarray(Wout, np.float32), np.asarray(bout, np.float32))

    xT = np.zeros((128, NXP), np.float32)
    xT[:, :N] = x.T
    ident = np.eye(128, dtype=np.float32)

    in_maps = []
    for c in range(NCORES):
        xs = np.zeros((128, NPAD), np.float32)
        xs[:, :NLOC] = x[c * NLOC + perms[c]].T
        in_maps.append({
            "xT": xT, "xs": xs, "rhs1": rhs1, "wdx1": wdx1, "rhs2": rhs2,
            "woutd": woutd, "b1t": b1t, "bft": bft, "ident": ident,
            "i1lo": np.ascontiguousarray(meta1["idx_lo"][c]),
            "i1hi": np.ascontiguousarray(meta1["idx_hi"][c]),
            "i2lo": np.ascontiguousarray(meta2["idx_lo"][c]),
            "i2hi": np.ascontiguousarray(meta2["idx_hi"][c]),
            "m1": np.ascontiguousarray(meta1["mask"][c]),
            "m2": np.ascontiguousarray(meta2["mask"][c]),
        })

    trace = bool(int(os.environ.get("GAT_TRACE", "0")))
    res = run_bass_kernel_spmd(nc, in_maps, core_ids=list(range(NCORES)),
                               trace=trace)
    LAST_RESULT = res
    global LAST_NC, LAST_IN_MAPS
    LAST_NC, LAST_IN_MAPS = nc, in_maps

    out = np.empty((N, OUTF), np.float32)
    for c in range(NCORES):
        out[c * NLOC + perms[c]] = res.results[c]["outy"][:NLOC]
    return out



# revision 9
# speedup vs baseline: 4.0342x; 4.0342x over previous
"""GAT (2-layer, 4-head) Trainium2 Bass kernel, 8-core SPMD — v2.

Layer 1: no gather. Host lays out x[src] in (window, slot, lane) cell order;
device streams it sequentially and computes h1 + a_s1 per cell on PE
(h1 = x_cell @ W1 with the attention vectors folded into extra columns).
Layer 2: dst-major dma_gather from the AllGather'd T2 row table, with a
single merged slot pool per window (windows sorted by total degree, so
in-window slot-count spread ~1) and mid-base signed int16 indices
(base row 17408) to address all 50176 rows in one gather per window.
Softmax + weighted aggregation run dst-major on DVE as in v1.
"""

import os
import numpy as np
from contextlib import ExitStack

import concourse.bass as bass
import concourse.tile as tile
from concourse import bacc, mybir
from concourse.bass_utils import run_bass_kernel_spmd

# problem constants (hardcoded per contest contract)
N = 50000
E = 1600000
HEADS = 4
HID = 32
INF = 128
OUTF = 8
NCORES = 8
NLOC = N // NCORES            # 6250 dst per core
WPC = (NLOC + 127) // 128     # 49 windows per core
NPAD = WPC * 128              # 6272
TB2 = NCORES * NPAD           # 50176 rows in layer-2 table
BASE = 17408                  # mid-base for signed int16 gather indices
ROWW = 256                    # fp16 words per T2 row (512 B)
L1W = 136                     # fp16 words per L1 cell row (h 128 + a_s 8)
DCAP = 24                     # slot-chunk for multiply/reduce working tile
PIECE = 16                    # stream chunks (of 128 cells) per DMA piece
NEG = -1e30

F32 = mybir.dt.float32
F16 = mybir.dt.float16
I16 = mybir.dt.int16

_CACHE = {}
LAST_RESULT = None
LAST_NC = None
LAST_IN_MAPS = None


# ----------------------------------------------------------------------------
# host-side graph preprocessing
# ----------------------------------------------------------------------------

def _host_prep(edge_index):
    srcs = np.concatenate([edge_index[0], np.arange(N)]).astype(np.int64)
    dsts = np.concatenate([edge_index[1], np.arange(N)]).astype(np.int64)
    ne = srcs.shape[0]

    core = dsts // NLOC
    deg = np.bincount(dsts, minlength=N)

    perms = []
    pos = np.empty(N, np.int64)
    for c in range(NCORES):
        p = np.argsort(-deg[c * NLOC:(c + 1) * NLOC], kind="stable")
        perms.append(p)
        pos[c * NLOC + p] = np.arange(NLOC)

    wpos = pos[dsts]
    w_e = wpos // 128
    lane_e = wpos % 128

    # slot j within each (core, dst) lane, in edge order
    key = core * NLOC + wpos
    order = np.argsort(key, kind="stable")
    ks = key[order]
    change = np.r_[True, ks[1:] != ks[:-1]]
    startpos = np.flatnonzero(change)
    gid = np.cumsum(change) - 1
    j_sorted = np.arange(ne) - startpos[gid]
    j = np.empty(ne, np.int64)
    j[order] = j_sorted

    degs = np.zeros((NCORES, NPAD), np.int64)
    for c in range(NCORES):
        degs[c, :NLOC] = deg[c * NLOC + perms[c]]
    d_w = degs.reshape(NCORES, WPC, 128).max(axis=(0, 2))
    d_w[WPC - 1] = max(int(d_w[WPC - 1]), 1)
    cs = np.r_[0, np.cumsum(d_w)]
    tot = int(cs[-1])
    dmax = int(d_w.max())
    assert dmax <= 80, dmax

    cellpos = (cs[w_e] + j) * 128 + lane_e  # flat cell column per edge

    src_cell = np.full((NCORES, tot * 128), -1, np.int64)
    src_cell[core, cellpos] = srcs

    maskf = np.full((NCORES, 128, tot), NEG, np.float32)
    maskf[core, lane_e, cs[w_e] + j] = 0.0
    # pad dst lanes (last window, pos >= NLOC): open slot 0 so denom stays finite
    maskf[:, (NLOC % 128):, int(cs[WPC - 1])] = 0.0

    t2row = (np.arange(N) // NLOC) * NPAD + pos

    idxv = np.zeros((NCORES, tot * 128), np.int16)
    real = src_cell >= 0
    idxv[real] = (t2row[src_cell[real]] - BASE).astype(np.int16)

    # trim-safety: the LAST index of each per-window gather (slot d-1, lane
    # 127) must be >= 0, else Q7 ucode trims it (and possibly real cells
    # before it). Pads are 0 (safe). For a real negative cell, swap slots
    # within the lane to put a nonneg-index cell last.
    for c in range(NCORES):
        for w in range(WPC):
            d = int(d_w[w])
            last = (int(cs[w]) + d - 1) * 128 + 127
            if idxv[c, last] >= 0:
                continue
            lanecols = [(int(cs[w]) + jj) * 128 + 127 for jj in range(d)]
            swapped = False
            for col in lanecols[:-1]:
                if idxv[c, col] >= 0:
                    for arr in (idxv, src_cell):
                        arr[c, col], arr[c, last] = arr[c, last], arr[c, col]
                    m_a = (128 * maskf.shape[2])  # not used; masks swap below
                    jj_a = col // 128 - int(cs[w])
                    jj_b = d - 1
                    ma = maskf[c, 127, int(cs[w]) + jj_a]
                    maskf[c, 127, int(cs[w]) + jj_a] = maskf[c, 127, int(cs[w]) + jj_b]
                    maskf[c, 127, int(cs[w]) + jj_b] = ma
                    swapped = True
                    break
            assert swapped, f"unfixable trim boundary core {c} window {w}"

    # wrap idx into dma_gather layout [128, 8*tot] (16-partition wrap, 8x rep)
    idxw = np.zeros((NCORES, 128, 8 * tot), np.int16)
    for w in range(WPC):
        d = int(d_w[w])
        cw = int(cs[w])
        blk = idxv[:, cw * 128:(cw + d) * 128]                 # [NC, d*128]
        blk = blk.reshape(NCORES, -1, 16).transpose(0, 2, 1)   # [NC, 16, 8d]
        idxw[:, :, 8 * cw: 8 * (cw + d)] = np.tile(blk, (1, 8, 1))

    return dict(d_w=d_w, cs=cs, tot=tot, dmax=dmax,
                src_cell=src_cell, mask=maskf, idxw=idxw), perms


def _fold_weights(W1, att_src1, att_dst1, b1, W2, att_src2, att_dst2, b2, Wout, bout):
    # device feature order is head-interleaved: dev k = c*4 + h <-> ref h*32 + c
    perm = np.array([h * 32 + c for c in range(HID) for h in range(HEADS)])

    def vec(att):  # [HEADS, HID] -> [128, 4] fold in dev space
        z = np.zeros((INF, HEADS), np.float32)
        k = np.arange(INF)
        z[k, k % HEADS] = att[k % HEADS, k // HEADS]
        return z

    W1d = W1[:, perm]
    rhs1 = np.concatenate([W1d, W1d @ vec(att_src1)], axis=1).astype(np.float16)  # [128,132]
    wdx1 = (W1d @ vec(att_dst1)).astype(np.float32)                               # [128,4]
    W2d = W2[perm][:, perm]
    rhs2 = np.concatenate([W2d, W2d @ vec(att_src2), W2d @ vec(att_dst2)], axis=1).astype(np.float32)
    woutd = Wout[perm].astype(np.float32)                                          # [128,8]
    b1t = np.tile(b1[perm].astype(np.float32), (128, 1))                           # [128,128]
    bf = (b2 @ Wout + bout).astype(np.float32)
    bft = np.tile(bf, (128, 1))                                                    # [128,8]
    return rhs1, wdx1, rhs2, woutd, b1t, bft


# ----------------------------------------------------------------------------
# device program
# ----------------------------------------------------------------------------

def _softmax_agg(nc, pools, w, d, xg, roww, mk_t, cs, ad_tile, den_tile, aggn_cb):
    """Per-window softmax over incoming edges + weighted aggregation.
    xg: [128, d, roww] fp16 with h at [0:128] and a_s f32 at [128:136]."""
    spool, wpool = pools["s"], pools["w"]
    cw = int(cs[w])

    as_view = xg[:, :, 128:136].bitcast(F32)                      # [128,d,4]
    lp = spool.tile([128, d, 4], F32, tag="lp")
    nc.vector.tensor_add(lp[:], as_view,
                         ad_tile[:, 4 * w:4 * w + 4].unsqueeze(1).broadcast_to([128, d, 4]))
    nc.vector.tensor_add(lp[:], lp[:],
                         mk_t[:, cw:cw + d].unsqueeze(2).broadcast_to([128, d, 4]))
    ll = spool.tile([128, d, 4], F32, tag="ll")
    nc.vector.scalar_tensor_tensor(ll[:], lp[:], 0.2, lp[:],
                                   mybir.AluOpType.mult, mybir.AluOpType.max)
    ew = spool.tile([128, d, 4], F32, tag="ew")
    nc.scalar.activation(ew[:], ll[:], mybir.ActivationFunctionType.Exp)

    nc.vector.tensor_reduce(den_tile[:, 4 * w:4 * w + 4], ew[:].transpose([0, 2, 1]),
                            mybir.AxisListType.X, mybir.AluOpType.add)

    agg = spool.tile([128, 128], F32, tag="agg")
    first = True
    for j0 in range(0, d, DCAP):
        dc = min(DCAP, d - j0)
        wm = wpool.tile([128, dc, 128], F32, tag="wm")
        xv = xg[:, j0:j0 + dc, 0:128].rearrange("p j (c h) -> p j c h", h=HEADS)
        eb = ew[:, j0:j0 + dc, :].unsqueeze(2).broadcast_to([128, dc, HID, HEADS])
        nc.vector.tensor_mul(wm[:].rearrange("p j (c h) -> p j c h", h=HEADS), xv, eb)
        if first:
            nc.vector.tensor_reduce(agg[:], wm[:].transpose([0, 2, 1]),
                                    mybir.AxisListType.X, mybir.AluOpType.add)
            first = False
        else:
            ac = spool.tile([128, 128], F32, tag="aggc")
            nc.vector.tensor_reduce(ac[:], wm[:].transpose([0, 2, 1]),
                                    mybir.AxisListType.X, mybir.AluOpType.add)
            nc.vector.tensor_add(agg[:], agg[:], ac[:])

    rec = spool.tile([128, 4], F32, tag="rec")
    nc.vector.reciprocal(rec[:], den_tile[:, 4 * w:4 * w + 4])
    aggn = spool.tile([128, 128], F32, tag="aggn")
    nc.vector.tensor_mul(aggn[:].rearrange("p (c h) -> p c h", h=HEADS),
                         agg[:].rearrange("p (c h) -> p c h", h=HEADS),
                         rec[:].unsqueeze(1).broadcast_to([128, HID, HEADS]))
    aggn_cb(w, aggn)


def _build_program(meta):
    d_w, cs, tot = meta["d_w"], meta["cs"], meta["tot"]
    npieces = (tot + PIECE - 1) // PIECE

    nc = bacc.Bacc("TRN2", num_devices=NCORES)

    xeT = nc.dram_tensor("xeT", [128, tot * 128], F16, kind="ExternalInput")
    xs = nc.dram_tensor("xs", [128, NPAD], F32, kind="ExternalInput")
    rhs1_h = nc.dram_tensor("rhs1", [128, 132], F16, kind="ExternalInput")
    wdx1_h = nc.dram_tensor("wdx1", [128, 4], F32, kind="ExternalInput")
    rhs2_h = nc.dram_tensor("rhs2", [128, 136], F32, kind="ExternalInput")
    wout_h = nc.dram_tensor("woutd", [128, 8], F32, kind="ExternalInput")
    b1t_h = nc.dram_tensor("b1t", [128, 128], F32, kind="ExternalInput")
    bft_h = nc.dram_tensor("bft", [128, 8], F32, kind="ExternalInput")
    ident_h = nc.dram_tensor("ident", [128, 128], F32, kind="ExternalInput")
    i2_h = nc.dram_tensor("i2", [128, 8 * tot], I16, kind="ExternalInput")
    mk_h = nc.dram_tensor("mk", [128, tot], F32, kind="ExternalInput")

    outy = nc.dram_tensor("outy", [NPAD, OUTF], F32, kind="ExternalOutput")

    AGIN = nc.dram_tensor("AGIN", [NPAD, ROWW], F16, kind="Internal")
    T2 = nc.dram_tensor("T2", [TB2, ROWW], F16, kind="Internal")

    with ExitStack() as ctx:
        tc = ctx.enter_context(tile.TileContext(nc))
        cpool = ctx.enter_context(tc.tile_pool(name="consts", bufs=1))
        pers = ctx.enter_context(tc.tile_pool(name="pers", bufs=1))
        strpool = ctx.enter_context(tc.tile_pool(name="stream", bufs=3))
        gpool = ctx.enter_context(tc.tile_pool(name="gather", bufs=2))
        wpool = ctx.enter_context(tc.tile_pool(name="work", bufs=2))
        spool = ctx.enter_context(tc.tile_pool(name="small", bufs=3))
        pspool = ctx.enter_context(tc.tile_pool(name="ps", bufs=3, space="PSUM"))
        ptpool = ctx.enter_context(tc.tile_pool(name="pt", bufs=2, space="PSUM"))
        pools = {"s": spool, "w": wpool}

        def const(h, shape, dtype=F32, tag=None):
            t = cpool.tile(shape, dtype, tag=tag)
            nc.sync.dma_start(t[:], h[:])
            return t

        rhs1_t = const(rhs1_h, [128, 132], F16, tag="rhs1")
        wdx1_t = const(wdx1_h, [128, 4], tag="wdx1")
        rhs2_t = const(rhs2_h, [128, 136], tag="rhs2")
        wout_t = const(wout_h, [128, 8], tag="wout")
        b1t_t = const(b1t_h, [128, 128], tag="b1t")
        bft_t = const(bft_h, [128, 8], tag="bft")
        ident_t = const(ident_h, [128, 128], tag="identc")
        i2_t = const(i2_h, [128, 8 * tot], I16, tag="i2")
        mk_t = const(mk_h, [128, tot], tag="mk")

        ad1 = pers.tile([128, 4 * WPC], F32)
        ad2 = pers.tile([128, 4 * WPC], F32)
        den1 = pers.tile([128, 4 * WPC], F32)
        den2 = pers.tile([128, 4 * WPC], F32)
        fin = pers.tile([128, OUTF * WPC], F32)

        # ---- a_d1 for owned (sorted) nodes ----
        for w in range(WPC):
            xs_w = spool.tile([128, 128], F32, tag="xsw")
            nc.sync.dma_start(xs_w[:], xs[:, w * 128:(w + 1) * 128])
            ps4 = ptpool.tile([128, 136], F32, tag="psb")
            nc.tensor.matmul(ps4[:, 0:4], xs_w[:], wdx1_t[:], start=True, stop=True)
            nc.vector.tensor_copy(ad1[:, 4 * w:4 * w + 4], ps4[:, 0:4])

        stop = os.environ.get("GAT_STOP", "full")

        # ---- layer 1: stream x[src] cells, matmul h1+a_s1, softmax-agg ----
        def tail1(w, aggn):
            t = spool.tile([128, 128], F32, tag="t1t")
            nc.vector.tensor_add(t[:], aggn[:], b1t_t[:])
            mn = spool.tile([128, 128], F32, tag="t1m")
            nc.vector.tensor_scalar_min(mn[:], t[:], 0.0)
            ex = spool.tile([128, 128], F32, tag="t1e")
            nc.scalar.activation(ex[:], mn[:], mybir.ActivationFunctionType.Exp)
            x2 = spool.tile([128, 128], F32, tag="t1x")
            nc.vector.scalar_tensor_tensor(x2[:], t[:], 0.0, ex[:],
                                           mybir.AluOpType.max, mybir.AluOpType.add)
            nc.vector.tensor_scalar_sub(x2[:], x2[:], 1.0)
            x2t_ps = ptpool.tile([128, 128], F32, tag="tr")
            nc.tensor.transpose(x2t_ps[:], x2[:], ident_t[:])
            x2t = spool.tile([128, 128], F32, tag="t1xt")
            nc.vector.tensor_copy(x2t[:], x2t_ps[:])
            ps = ptpool.tile([128, 136], F32, tag="psb")
            nc.tensor.matmul(ps[:], x2t[:], rhs2_t[:], start=True, stop=True)
            rowt = spool.tile([128, 128], F32, tag="rowt")
            nc.scalar.copy(rowt[:].bitcast(F16)[:, 0:128], ps[:, 0:128])
            nc.vector.tensor_copy(rowt[:, 64:68], ps[:, 128:132])
            nc.vector.memset(rowt[:, 68:128], 0.0)
            nc.vector.tensor_copy(ad2[:, 4 * w:4 * w + 4], ps[:, 132:136])
            nc.sync.dma_start(
                AGIN[w * 128:(w + 1) * 128, :].rearrange("(a p) r -> p a r", p=128),
                rowt[:].bitcast(F16).rearrange("p (a r) -> p a r", a=1))

        if stop != "a":
            # stream pieces: piece p covers chunks [p*PIECE, (p+1)*PIECE)
            piece_tiles = {}

            def get_piece(p):
                if p in piece_tiles:
                    return piece_tiles[p]
                k = min(PIECE, tot - p * PIECE)
                pt = strpool.tile([128, 128 * PIECE], F16, tag="xep")
                nc.sync.dma_start(pt[:, 0:128 * k],
                                  xeT[:, p * PIECE * 128:(p * PIECE + k) * 128])
                piece_tiles[p] = pt
                return pt

            for w in range(WPC):
                d = int(d_w[w])
                cw = int(cs[w])
                xg = gpool.tile([128, d, L1W], F16, tag="xg1")
                # h1 + a_s1 per slot-column, batched 3 chunks per PSUM tile
                for j0 in range(0, d, 3):
                    bn = min(3, d - j0)
                    ps = pspool.tile([128, 132 * bn], F32, tag="mm")
                    for k in range(bn):
                        g = cw + j0 + k
                        pt = get_piece(g // PIECE)
                        off = (g % PIECE) * 128
                        nc.tensor.matmul(ps[:, 132 * k:132 * (k + 1)],
                                         pt[:, off:off + 128], rhs1_t[:],
                                         start=True, stop=True)
                    nc.scalar.copy(
                        xg[:, j0:j0 + bn, 0:128],
                        ps[:].rearrange("p (b c) -> p b c", b=bn)[:, :, 0:128])
                    nc.vector.tensor_copy(
                        xg[:, j0:j0 + bn, 128:136].bitcast(F32),
                        ps[:].rearrange("p (b c) -> p b c", b=bn)[:, :, 128:132])
                _softmax_agg(nc, pools, w, d, xg, L1W, mk_t, cs, ad1, den1, tail1)

        if stop in ("a", "l1"):
            nc.vector.memset(fin[:], 0.0)
        else:
            nc.gpsimd.collective_compute(
                "AllGather", mybir.AluOpType.bypass,
                replica_groups=[list(range(NCORES))],
                ins=[AGIN[:].opt()], outs=[T2[:].opt()])

        # ---- layer 2: dma_gather rows from T2, softmax-agg, project ----
        def tail2(w, aggn):
            at_ps = ptpool.tile([128, 128], F32, tag="tr")
            nc.tensor.transpose(at_ps[:], aggn[:], ident_t[:])
            at = spool.tile([128, 128], F32, tag="t2at")
            nc.vector.tensor_copy(at[:], at_ps[:])
            ps8 = ptpool.tile([128, 136], F32, tag="psb")
            nc.tensor.matmul(ps8[:, 0:8], at[:], wout_t[:], start=True, stop=True)
            nc.vector.tensor_add(fin[:, OUTF * w:OUTF * (w + 1)], ps8[:, 0:8], bft_t[:])

        if stop == "ag":
            nc.vector.memset(fin[:], 0.0)
        if stop == "full":
            for w in range(WPC):
                d = int(d_w[w])
                cw = int(cs[w])
                xg = gpool.tile([128, d, ROWW], F16, tag="xg2")
                nc.gpsimd.dma_gather(xg[:], T2[BASE:, :],
                                     i2_t[:, 8 * cw:8 * (cw + d)],
                                     128 * d, 128 * d, ROWW, single_packet=False)
                _softmax_agg(nc, pools, w, d, xg, ROWW, mk_t, cs, ad2, den2, tail2)

        nc.sync.dma_start(outy[:].rearrange("(a p) r -> p a r", p=128),
                          fin[:].rearrange("p (a r) -> p a r", a=WPC))

    nc.compile()
    return nc


# ----------------------------------------------------------------------------
# entry point
# ----------------------------------------------------------------------------

def kernel(x, edge_index, W1, att_src1, att_dst1, b1, W2, att_src2, att_dst2,
           b2, Wout, bout):
    global LAST_RESULT, LAST_NC, LAST_IN_MAPS
    x = np.asarray(x, np.float32)
    edge_index = np.asarray(edge_index)

    ck = hash(edge_index.tobytes())
    if ck not in _CACHE:
        meta, perms = _host_prep(edge_index)
        nc = _build_program(meta)
        _CACHE.clear()
        _CACHE[ck] = (meta, perms, nc)
    meta, perms, nc = _CACHE[ck]
    tot = meta["tot"]

    rhs1, wdx1, rhs2, woutd, b1t, bft = _fold_weights(
        np.asarray(W1, np.float32), np.asarray(att_src1, np.float32),
        np.asarray(att_dst1, np.float32), np.asarray(b1, np.float32),
        np.asarray(W2, np.float32), np.asarray(att_src2, np.float32),
        np.asarray(att_dst2, np.float32), np.asarray(b2, np.float32),
        np.asarray(Wout, np.float32), np.asarray(bout, np.float32))

    ident = np.eye(128, dtype=np.float32)
    x16 = x.astype(np.float16)

    in_maps = []
    for c in range(NCORES):
        sc = meta["src_cell"][c]
        xeT = np.zeros((tot * 128, 128), np.float16)
        real = sc >= 0
        xeT[real] = x16[sc[real]]
        xs = np.zeros((128, NPAD), np.float32)
        xs[:, :NLOC] = x[c * NLOC + perms[c]].T
        in_maps.append({
            "xeT": np.ascontiguousarray(xeT.T), "xs": xs, "rhs1": rhs1,
            "wdx1": wdx1, "rhs2": rhs2, "woutd": woutd, "b1t": b1t, "bft": bft,
            "ident": ident,
            "i2": np.ascontiguousarray(meta["idxw"][c]),
            "mk": np.ascontiguousarray(meta["mask"][c]),
        })

    trace = bool(int(os.environ.get("GAT_TRACE", "0")))
    res = run_bass_kernel_spmd(nc, in_maps, core_ids=list(range(NCORES)),
                               trace=trace)
    LAST_RESULT = res
    LAST_NC, LAST_IN_MAPS = nc, in_maps

    out = np.empty((N, OUTF), np.float32)
    for c in range(NCORES):
        out[c * NLOC + perms[c]] = res.results[c]["outy"][:NLOC]
    return out
